# revision 12
# baseline (speedup 1.0000x reference)
"""ConvTranspose2d (16,256,32,32) -> (16,128,66,66), stride 2, 4x4 kernel.

Strategy: data-parallel over batch, 2 images per core on 8 NeuronCores.

Math: y[b,co,2m+p,2n+q] = bias[co]
        + sum_{i,j in {0,1}} sum_ci x[b,ci,m-i,n-j] * w[ci,co,p+2i,q+2j]
for parity class (p,q) in {0,1}^2, m,n in [0,33).

Per image and parity class: output subgrid [128co x 33 x 33] computed as
3 row-chunks; each chunk is one PSUM accumulation group of 8 bf16
matmuls (2 ci-chunks x 4 taps (i,j)), K=128, M=128, N=R*34, accumulated
in fp32 PSUM.  Inputs ride in bf16 (rel err ~2.4e-3, threshold 2e-2),
halving input HBM traffic.  Shifted taps read a zero-padded 34x34 SBUF
copy of x (padded host-side); the pad column rides along in the matmul
free dim and is dropped on drain.  PSUM->SBUF drain is a DVE
tensor_scalar_add fusing the bias add and the parity de-interleave.

Overlap choices: the 16 DMA channels are shared round-robin across
the three issue queues (sync/scalar HW rings + gpsimd software DGE),
totalling ~200GB/s inbound, so the input plan balances BYTES per queue
and relies on per-queue FIFO for priority: phase 1 is x0 halves +
class-(0,0) weights (one third on each queue), phase 2 queues the
remaining weights and x1 right behind, sized so every class lands
before the full-rate stream needs it.  Image-0's y leaves as partition
halves on the HW rings (idle mid-kernel); image-1's output leaves as
three row bands on gpsimd, the last band only 10 rows so little trails
the final matmul.  A bf16 dummy-matmul warmup starts immediately and
bridges until real data lands (HAM runs the PE at 1.2GHz until ~3.5us
of sustained activity, and any idle PE gap re-throttles to half
clock and restarts the clock).
"""

import numpy as np
import ml_dtypes

import concourse.bass as bass
import concourse.bacc as bacc
import concourse.tile as tile
from concourse import mybir
from concourse.bass_utils import run_bass_kernel_spmd

N_CORES = 8
B_PER = 2  # images per core

F32 = mybir.dt.float32
BF16 = mybir.dt.bfloat16

PW = 34            # padded x width (32 + 1 left + 1 right)
XLEN = PW * PW     # 1156 padded x elems per partition
XPAD = 1160        # sbuf/dram x free size (AP slack for the last chunk)

# (m0, R) parity-row chunks; rows m0..m0+R-1 of the 33-row parity grid.
# One PSUM accumulation group per chunk: N = R*34 fp32 <= 2KB PSUM bank.
CHUNKS0 = [(0, 14), (14, 14), (28, 5)]
# image 1 drains through progressively smaller output bands so the
# last band's DMA is tiny and almost nothing trails the final matmul
CHUNKS1 = [(0, 14), (14, 9), (23, 5), (28, 5)]

# partition split across the two HW rings (must stay 32-aligned --
# unaligned partition bases hit a pathologically slow DMA path)
PSPLIT = 64


def _emit_group(nc, ps, wt, xv, p, q, m0, R):
    """One PSUM accumulation group: 8 matmuls for class (p,q), one chunk."""
    nf = R * PW
    k = 0
    for c in range(2):
        for i in range(2):
            for j in range(2):
                off = (m0 - i + 1) * PW + (1 - j)
                nc.tensor.matmul(
                    ps[:],
                    wt[:, p, q, c, i, j, :],
                    xv[:, c, off:off + nf],
                    start=(k == 0),
                    stop=(k == 7),
                )
                k += 1


def build_nc(debug: bool = False) -> bass.Bass:
    nc = bacc.Bacc("TRN2", target_bir_lowering=False, debug=debug,
                   num_devices=N_CORES)

    # x arrives host-padded bf16: [img, ci'=128, c=2, 34*34+tail] flat
    x_d = nc.declare_dram_parameter("x", [B_PER, 128, 2, XPAD], BF16,
                                    isOutput=False)
    # w layout: [ci'=128, p, q, c, i, j, co] -- a whole class is one
    # contiguous 2KB run per partition, so one DMA per class is efficient
    w_d = nc.declare_dram_parameter("w", [128, 2, 2, 2, 2, 2, 128], BF16,
                                    isOutput=False)
    b_d = nc.declare_dram_parameter("b", [128, 1], F32, isOutput=False)
    y_d = nc.declare_dram_parameter("y", [B_PER, 128, 66, 66], F32,
                                    isOutput=True)

    with tile.TileContext(nc) as tc:
        with (
            tc.tile_pool(name="wp", bufs=1) as wpool,
            tc.tile_pool(name="bp", bufs=1) as bpool,
            tc.tile_pool(name="xp", bufs=B_PER) as xpool,
            tc.tile_pool(name="yp", bufs=1) as ypool,
            tc.tile_pool(name="ybp", bufs=4) as bandpool,
            tc.tile_pool(name="wu", bufs=1) as wupool,
            tc.tile_pool(name="ps", bufs=7, space="PSUM") as ppool,
            tc.tile_pool(name="pw", bufs=1, space="PSUM") as warmpool,
        ):
            # PE warm-up on gpsimd-memset garbage (gpsimd runs user code
            # first, so the warmup starts ~1us earlier than via vector).
            # HAM starts the PE at 1.2GHz and unthrottles after ~3.5us of
            # sustained activity; the dummies bridge until real data lands.
            wub = wupool.tile([128, 512], BF16)
            nc.gpsimd.memset(wub[:], 0.0)
            wps = warmpool.tile([128, 512], F32)
            for _ in range(10):
                nc.tensor.matmul(wps[:], wub[:, 0:128], wub[:],
                                 start=True, stop=True)

            # bias rides gpsimd (the output queue, idle at start) so it
            # lands before the first drain without delaying inputs
            bt = bpool.tile([128, 1], F32)
            nc.gpsimd.dma_start(out=bt[:], in_=b_d[:])

            wt = wpool.tile([128, 2, 2, 2, 2, 2, 128], BF16)
            xt = [xpool.tile([128, 2, XPAD], BF16, name=f"x{img}", tag="xt")
                  for img in range(B_PER)]

            # inputs ride mostly the two HW rings (sync ~130GB/s,
            # scalar ~85GB/s), FIFO in consumption order; gpsimd's software
            # DGE is slow inbound (~40GB/s) but idle early, so it carries
            # the second halves of the first two weight classes, letting
            # the first matmul go as soon as x0 lands
            nc.sync.dma_start(out=xt[0][0:64], in_=x_d[0][0:64])
            nc.scalar.dma_start(out=xt[0][64:128], in_=x_d[0][64:128])
            nc.gpsimd.dma_start(out=wt[64:128, 0, 0], in_=w_d[64:128, 0, 0])
            nc.gpsimd.dma_start(out=wt[64:128, 0, 1], in_=w_d[64:128, 0, 1])
            nc.sync.dma_start(out=wt[0:64, 0, 0], in_=w_d[0:64, 0, 0])
            nc.sync.dma_start(out=wt[0:64, 0, 1], in_=w_d[0:64, 0, 1])
            nc.sync.dma_start(out=wt[0:64, 1], in_=w_d[0:64, 1])
            nc.scalar.dma_start(out=wt[64:128, 1], in_=w_d[64:128, 1])
            nc.sync.dma_start(out=xt[1][0:64], in_=x_d[1][0:64])
            nc.scalar.dma_start(out=xt[1][64:128], in_=x_d[1][64:128])

            def drain(ps, R, out_view):
                nc.vector.tensor_scalar_add(
                    out_view,
                    ps[:].rearrange("p (m n) -> p m n", n=PW)[:, :, 0:33],
                    bt[:],
                )

            # ---- image 0: class-major (w classes stream in one at a
            # time); y leaves as row bands, each gated only on the drains
            # that wrote those rows, so output starts before the last
            # class finishes ----
            yt = ypool.tile([128, 66, 66], F32)
            for p in range(2):
                for q in range(2):
                    for m0, R in CHUNKS0:
                        ps = ppool.tile([128, R * PW], F32)
                        _emit_group(nc, ps, wt, xt[0], p, q, m0, R)
                        drain(ps, R, yt[:, p::2, q::2][:, m0:m0 + R, :])
            for bi, (m0, R) in enumerate(CHUNKS0):
                rows = slice(2 * m0, 2 * (m0 + R))
                if bi == 1:
                    nc.sync.dma_start(out=y_d[0][0:PSPLIT, rows],
                                      in_=yt[0:PSPLIT, rows])
                    nc.scalar.dma_start(out=y_d[0][PSPLIT:128, rows],
                                        in_=yt[PSPLIT:128, rows])
                else:
                    nc.gpsimd.dma_start(out=y_d[0][:, rows], in_=yt[:, rows])

            # ---- image 1: band-major; banded output DMAs.  The first
            # two bands ride gpsimd (fast outbound) while y0 drains the HW
            # rings; the small last band is halved onto the HW rings,
            # idle again by then, so almost nothing trails the last matmul.
            for bi, (m0, R) in enumerate(CHUNKS1):
                band = bandpool.tile([128, 2 * R, 66], F32)
                for p in range(2):
                    for q in range(2):
                        ps = ppool.tile([128, R * PW], F32)
                        _emit_group(nc, ps, wt, xt[1], p, q, m0, R)
                        drain(ps, R, band[:, p::2, q::2])
                y_view = y_d[1][:, 2 * m0:2 * (m0 + R), :]
                if bi != 2:
                    nc.gpsimd.dma_start(out=y_view, in_=band[:])
                else:
                    nc.sync.dma_start(out=y_view[0:PSPLIT], in_=band[0:PSPLIT])
                    nc.scalar.dma_start(out=y_view[PSPLIT:128],
                                        in_=band[PSPLIT:128])

    nc.compile()
    return nc


_nc_cache = None


def _get_nc():
    global _nc_cache
    if _nc_cache is None:
        _nc_cache = build_nc()
    return _nc_cache


def make_in_maps(x: np.ndarray, weight: np.ndarray, bias: np.ndarray):
    # w[ci,co,kh,kw] -> [ci', p, q, c, i, j, co]
    w7 = (
        np.asarray(weight, dtype=np.float32)
        .reshape(2, 128, 128, 2, 2, 2, 2)      # [c, ci', co, i, p, j, q]
        .transpose(1, 4, 6, 0, 3, 5, 2)        # -> [ci', p, q, c, i, j, co]
    )
    w_host = np.ascontiguousarray(w7.astype(ml_dtypes.bfloat16))
    b_host = np.ascontiguousarray(
        np.asarray(bias, dtype=np.float32).reshape(128, 1)
    )
    x = np.asarray(x, dtype=np.float32)
    # host-side zero-pad into the 34x34(+tail) layout, ci split [c, ci']
    # transposed to [ci', c], bf16
    xpad = np.zeros((16, 2, 128, XPAD), dtype=np.float32)
    xpad[:, :, :, :XLEN].reshape(16, 2, 128, PW, PW)[:, :, :, 1:33, 1:33] = (
        x.reshape(16, 2, 128, 32, 32)
    )
    xpad = np.ascontiguousarray(
        xpad.transpose(0, 2, 1, 3).astype(ml_dtypes.bfloat16)
    )
    return [
        {
            "x": xpad[B_PER * i:B_PER * (i + 1)],
            "w": w_host,
            "b": b_host,
        }
        for i in range(N_CORES)
    ]


def kernel(x: np.ndarray, weight: np.ndarray, bias: np.ndarray) -> np.ndarray:
    nc = _get_nc()
    in_maps = make_in_maps(x, weight, bias)
    res = run_bass_kernel_spmd(nc, in_maps, list(range(N_CORES)))
    out = np.concatenate([r["y"] for r in res.results], axis=0)
    return np.ascontiguousarray(out.astype(np.float32, copy=False))


# revision 13
# speedup vs baseline: 1.1097x; 1.1097x over previous
"""ConvTranspose2d (16,256,32,32) -> (16,128,66,66), stride 2, 4x4 kernel.

Strategy: data-parallel over batch, 2 images per core on 8 NeuronCores.

Math: y[b,co,2m+p,2n+q] = bias[co]
        + sum_{i,j in {0,1}} sum_ci x[b,ci,m-i,n-j] * w[ci,co,p+2i,q+2j]
for parity class (p,q) in {0,1}^2, m,n in [0,33).

Per image and parity class: output subgrid [128co x 33 x 33] computed as
3 row-chunks; each chunk is one PSUM accumulation group of 8 bf16
matmuls (2 ci-chunks x 4 taps (i,j)), K=128, M=128, N=R*34, accumulated
in fp32 PSUM.  Inputs ride in bf16 (rel err ~2.4e-3, threshold 2e-2),
halving input HBM traffic.  Shifted taps read a zero-padded 34x34 SBUF
copy of x (padded host-side); the pad column rides along in the matmul
free dim and is dropped on drain.  PSUM->SBUF drain is a DVE
tensor_scalar_add fusing the bias add and the parity de-interleave.

Overlap choices: the 16 DMA channels are shared round-robin across
the three issue queues (sync/scalar HW rings + gpsimd software DGE),
totalling ~200GB/s inbound, so the input plan balances BYTES per queue
and relies on per-queue FIFO for priority: phase 1 is x0 halves +
class-(0,0) weights (one third on each queue), phase 2 queues the
remaining weights and x1 right behind, sized so every class lands
before the full-rate stream needs it.  Image-0's y leaves as partition
halves on the HW rings (idle mid-kernel); image-1's output leaves as
three row bands on gpsimd, the last band only 10 rows so little trails
the final matmul.  A bf16 dummy-matmul warmup starts immediately and
bridges until real data lands (HAM runs the PE at 1.2GHz until ~3.5us
of sustained activity, and any idle PE gap re-throttles to half
clock and restarts the clock).
"""

import numpy as np
import ml_dtypes

import concourse.bass as bass
import concourse.bacc as bacc
import concourse.tile as tile
from concourse import mybir
from concourse.bass_utils import run_bass_kernel_spmd

N_CORES = 8
B_PER = 2  # images per core

F32 = mybir.dt.float32
BF16 = mybir.dt.bfloat16

PW = 34            # padded x width (32 + 1 left + 1 right)
XLEN = PW * PW     # 1156 padded x elems per partition
XPAD = 1160        # sbuf/dram x free size (AP slack for the last chunk)

# (m0, R) parity-row chunks; rows m0..m0+R-1 of the 33-row parity grid.
# One PSUM accumulation group per chunk: N = R*34 fp32 <= 2KB PSUM bank.
CHUNKS0 = [(0, 14), (14, 14), (28, 5)]
# image 1 drains through progressively smaller output bands so the
# last band's DMA is tiny and almost nothing trails the final matmul
CHUNKS1 = [(0, 14), (14, 9), (23, 5), (28, 5)]

# partition split across the two HW rings (must stay 32-aligned --
# unaligned partition bases hit a pathologically slow DMA path)
PSPLIT = 64


def _emit_group(nc, ps, wt, xv, p, q, m0, R):
    """One PSUM accumulation group: 8 matmuls for class (p,q), one chunk."""
    nf = R * PW
    k = 0
    for c in range(2):
        for i in range(2):
            for j in range(2):
                off = (m0 - i + 1) * PW + (1 - j)
                nc.tensor.matmul(
                    ps[:],
                    wt[:, p, q, c, i, j, :],
                    xv[:, c, off:off + nf],
                    start=(k == 0),
                    stop=(k == 7),
                )
                k += 1


def build_nc(debug: bool = False) -> bass.Bass:
    nc = bacc.Bacc("TRN2", target_bir_lowering=False, debug=debug,
                   num_devices=N_CORES)

    # x arrives host-padded bf16: [img, ci'=128, c=2, 34*34+tail] flat
    x_d = nc.declare_dram_parameter("x", [B_PER, 128, 2, XPAD], BF16,
                                    isOutput=False)
    # w layout: [ci'=128, p, q, c, i, j, co] -- a whole class is one
    # contiguous 2KB run per partition, so one DMA per class is efficient
    w_d = nc.declare_dram_parameter("w", [128, 2, 2, 2, 2, 2, 128], BF16,
                                    isOutput=False)
    b_d = nc.declare_dram_parameter("b", [128, 1], F32, isOutput=False)
    y_d = nc.declare_dram_parameter("y", [B_PER, 128, 66, 66], F32,
                                    isOutput=True)

    with tile.TileContext(nc) as tc:
        with (
            tc.tile_pool(name="wp", bufs=1) as wpool,
            tc.tile_pool(name="bp", bufs=1) as bpool,
            tc.tile_pool(name="xp", bufs=B_PER) as xpool,
            tc.tile_pool(name="yp", bufs=1) as ypool,
            tc.tile_pool(name="ybp", bufs=4) as bandpool,
            tc.tile_pool(name="wu", bufs=1) as wupool,
            tc.tile_pool(name="ps", bufs=7, space="PSUM") as ppool,
            tc.tile_pool(name="pw", bufs=1, space="PSUM") as warmpool,
        ):
            # PE warm-up on gpsimd-memset garbage (gpsimd runs user code
            # first, so the warmup starts ~1us earlier than via vector).
            # HAM starts the PE at 1.2GHz and unthrottles after ~3.5us of
            # sustained activity; the dummies bridge until real data lands.
            wub = wupool.tile([128, 512], BF16)
            nc.gpsimd.memset(wub[:], 0.0)
            wps = warmpool.tile([128, 512], F32)
            for _ in range(13):
                nc.tensor.matmul(wps[:], wub[:, 0:128], wub[:],
                                 start=True, stop=True)

            # bias rides gpsimd (the output queue, idle at start) so it
            # lands before the first drain without delaying inputs
            bt = bpool.tile([128, 1], F32)
            nc.gpsimd.dma_start(out=bt[:], in_=b_d[:])

            wt = wpool.tile([128, 2, 2, 2, 2, 2, 128], BF16)
            xt = [xpool.tile([128, 2, XPAD], BF16, name=f"x{img}", tag="xt")
                  for img in range(B_PER)]

            # inputs ride only the two HW rings (gpsimd's software DGE
            # is pathologically slow inbound while the rings are active),
            # partition-halved, FIFO enforcing consumption order
            def dma_in(dst, src):
                nc.sync.dma_start(out=dst[0:PSPLIT], in_=src[0:PSPLIT])
                nc.scalar.dma_start(out=dst[PSPLIT:128], in_=src[PSPLIT:128])

            dma_in(xt[0][:], x_d[0])
            dma_in(wt[:, 0, 0], w_d[:, 0, 0])
            dma_in(wt[:, 0, 1], w_d[:, 0, 1])
            dma_in(wt[:, 1], w_d[:, 1])
            dma_in(xt[1][:], x_d[1])

            def drain(ps, R, out_view):
                nc.vector.tensor_scalar_add(
                    out_view,
                    ps[:].rearrange("p (m n) -> p m n", n=PW)[:, :, 0:33],
                    bt[:],
                )

            # ---- image 0: class-major (w classes stream in one at a
            # time); y leaves as row bands, each gated only on the drains
            # that wrote those rows, so output starts before the last
            # class finishes ----
            yt = ypool.tile([128, 66, 66], F32)
            for p in range(2):
                for q in range(2):
                    for m0, R in CHUNKS0:
                        ps = ppool.tile([128, R * PW], F32)
                        _emit_group(nc, ps, wt, xt[0], p, q, m0, R)
                        drain(ps, R, yt[:, p::2, q::2][:, m0:m0 + R, :])
            for bi, (m0, R) in enumerate(CHUNKS0):
                rows = slice(2 * m0, 2 * (m0 + R))
                if bi == 1:
                    nc.sync.dma_start(out=y_d[0][0:PSPLIT, rows],
                                      in_=yt[0:PSPLIT, rows])
                    nc.scalar.dma_start(out=y_d[0][PSPLIT:128, rows],
                                        in_=yt[PSPLIT:128, rows])
                else:
                    nc.gpsimd.dma_start(out=y_d[0][:, rows], in_=yt[:, rows])

            # ---- image 1: band-major; banded output DMAs.  The first
            # two bands ride gpsimd (fast outbound) while y0 drains the HW
            # rings; the small last band is halved onto the HW rings,
            # idle again by then, so almost nothing trails the last matmul.
            for bi, (m0, R) in enumerate(CHUNKS1):
                band = bandpool.tile([128, 2 * R, 66], F32)
                for p in range(2):
                    for q in range(2):
                        ps = ppool.tile([128, R * PW], F32)
                        _emit_group(nc, ps, wt, xt[1], p, q, m0, R)
                        drain(ps, R, band[:, p::2, q::2])
                y_view = y_d[1][:, 2 * m0:2 * (m0 + R), :]
                if bi != 2:
                    nc.gpsimd.dma_start(out=y_view, in_=band[:])
                else:
                    nc.sync.dma_start(out=y_view[0:PSPLIT], in_=band[0:PSPLIT])
                    nc.scalar.dma_start(out=y_view[PSPLIT:128],
                                        in_=band[PSPLIT:128])

    nc.compile()
    return nc


_nc_cache = None


def _get_nc():
    global _nc_cache
    if _nc_cache is None:
        _nc_cache = build_nc()
    return _nc_cache


def make_in_maps(x: np.ndarray, weight: np.ndarray, bias: np.ndarray):
    # w[ci,co,kh,kw] -> [ci', p, q, c, i, j, co]
    w7 = (
        np.asarray(weight, dtype=np.float32)
        .reshape(2, 128, 128, 2, 2, 2, 2)      # [c, ci', co, i, p, j, q]
        .transpose(1, 4, 6, 0, 3, 5, 2)        # -> [ci', p, q, c, i, j, co]
    )
    w_host = np.ascontiguousarray(w7.astype(ml_dtypes.bfloat16))
    b_host = np.ascontiguousarray(
        np.asarray(bias, dtype=np.float32).reshape(128, 1)
    )
    x = np.asarray(x, dtype=np.float32)
    # host-side zero-pad into the 34x34(+tail) layout, ci split [c, ci']
    # transposed to [ci', c], bf16
    xpad = np.zeros((16, 2, 128, XPAD), dtype=np.float32)
    xpad[:, :, :, :XLEN].reshape(16, 2, 128, PW, PW)[:, :, :, 1:33, 1:33] = (
        x.reshape(16, 2, 128, 32, 32)
    )
    xpad = np.ascontiguousarray(
        xpad.transpose(0, 2, 1, 3).astype(ml_dtypes.bfloat16)
    )
    return [
        {
            "x": xpad[B_PER * i:B_PER * (i + 1)],
            "w": w_host,
            "b": b_host,
        }
        for i in range(N_CORES)
    ]


def kernel(x: np.ndarray, weight: np.ndarray, bias: np.ndarray) -> np.ndarray:
    nc = _get_nc()
    in_maps = make_in_maps(x, weight, bias)
    res = run_bass_kernel_spmd(nc, in_maps, list(range(N_CORES)))
    out = np.concatenate([r["y"] for r in res.results], axis=0)
    return np.ascontiguousarray(out.astype(np.float32, copy=False))
